# revision 2
# baseline (speedup 1.0000x reference)
"""Trainium2 Bass kernel for Mistral-style MHA prefill (sparse_attention).

Problem: B=2, S=2048, DIM=4096, 32 q heads / 8 kv heads, head_dim=128,
sliding window 2048 (== S -> pure causal), RoPE, fp32 reference.

Sharding (8 cores): DP over batch (2) x TP over heads (4).  Core c = b*4+tp
handles batch b, q-heads [tp*8, tp*8+8), kv-heads [tp*2, tp*2+2).  wq/wk/wv
sharded column-wise, wo row-wise; wo partials are written bf16 and reduced
on the host in f32.

Design (878us baseline -> 740us):
  - scores computed TRANSPOSED: scT[k, q] = (K^T blk).T @ Q^T chunk, so the
    scalar-engine exp() writes P^T directly to SBUF -> no PE transposes of P
    and no PSUM->SBUF copies for them (-58us PE, -100us scalar/vector).
  - softmax row sums via a ones-column appended to V (PV rhs is [128, 129]);
    l = pa[:, 128]; no scalar accumulator reads (-95us scalar).
  - full-k scores blocks are computed in pairs into [128, 2, 512] PSUM tiles
    so exp() runs once per 1024 columns; diagonal (masked) blocks share
    tiles in pairs to halve exp WAR pressure on PSUM.
  - program-order interleaving: scores run 2 blocks ahead of PV (two P^T
    slots tagged by q-chunk parity); Q-proj chains of the next head-pair are
    emitted between scores and PV to pad exp latency; the last head-pair of
    half 0 is padded with half 1's first V-proj chains, and the last
    head-pair of half 1 with in-place A^T transposes of half-0 output rows.
  - per-head wq tiles (two tags) so a filler chain only waits on a 1.05MB
    weight DMA; x split into two 512-column tiles per half so the half-2
    prefetch starts as soon as the sc=0 readers retire.
  - startup DMAs in consumption order (V weights + first x granules
    interleaved) -> first matmul at ~14us instead of 47us.
  - V projection emits one 256-wide chain for both kv heads; wo phase
    pipelines remaining A^T transposes into the previous block's chains,
    stages full [128, 4096] bf16 rows, chunks the last row's DMA.
"""

import os
import sys

import numpy as np

for _p in ("/opt/trn_rl_repo",):
    if _p not in sys.path and os.path.isdir(_p):
        sys.path.insert(0, _p)

import ml_dtypes  # noqa: E402

import concourse.bass as bass  # noqa: E402
import concourse.mybir as mybir  # noqa: E402
import concourse.tile as tile  # noqa: E402
from concourse.bass_utils import run_bass_kernel_spmd  # noqa: E402

BF16 = ml_dtypes.bfloat16


def _install_drain_split_patch():
    """The pinned walrus rejects Tile's kernel-tail Drain when it carries more
    than ~2 semaphore waits.  Split the global drain's waits across trailing
    sync-engine nops (1 wait each)."""
    if getattr(tile.TileContext, "_drain_split_patched", False):
        return
    from concourse.vector_clock import ScopedClock

    limit = 1

    def _patched_dab(self, tick_clock, wait_clock):
        drain_inst = self.nc.sync.drain()
        raw = drain_inst.ins
        wait_clock.add_sem_waits(raw, ScopedClock({None: tick_clock.global_clock}))
        si = raw.sync_info
        waits = list(si.on_wait or [])
        if len(waits) > limit:
            si.on_wait = waits[:limit]
            for i in range(limit, len(waits), limit):
                nraw = self.nc.sync.nop().ins
                nsi = nraw.sync_info
                if nsi is None:
                    nraw.sync_info = mybir.SyncInfo(
                        on_wait=waits[i : i + limit], on_update=[]
                    )
                else:
                    nsi.on_wait = list(nsi.on_wait or []) + waits[i : i + limit]
        self.nc.all_engine_barrier()
        popped = self.nc._tile_sem_poison_stack.pop()
        assert popped is self._sem_poison
        self.nc.clear_and_free_semaphores(list(self.sems.allocated().values()))
        self.nc.all_engine_barrier()

    tile.TileContext._drain_and_barrier = _patched_dab
    tile.TileContext._drain_split_patched = True


_install_drain_split_patch()

P = 128
S = 2048
D = 4096
KO = D // P  # 32 contraction chunks
SH = S // 2  # half of the sequence
NH_L = 8  # q heads per core
NKV_L = 2  # kv heads per core
DH = 128
VW = 136  # per-(sb, g) V row stride (128 dh + 1 ones + pad)
SCALE = float(DH) ** -0.5
N_CORES = 8

_dt_f32 = mybir.dt.float32
_dt_bf16 = mybir.dt.bfloat16


def _emit(tc, aps):
    nc = tc.nc
    xr = aps["xT"].rearrange("(ko p) s -> p ko s", p=P)  # [128, 32, 2048]
    wqr = aps["wqT"].rearrange("(ko p) o -> p ko o", p=P)  # [128, 32, 1024]
    wkvr = aps["wkvT"].rearrange("(ko p) o -> p ko o", p=P)  # [128, 32, 512]
    wor = aps["woT"].rearrange("(ho p) e -> p ho e", p=P)  # [128, 8, 4096]
    out_ap = aps["out"]  # [2048, 4096] bf16

    from contextlib import ExitStack

    with ExitStack() as g:
        singles = g.enter_context(tc.tile_pool(name="singles", bufs=1))
        kv_pool = g.enter_context(tc.tile_pool(name="kvp", bufs=1))
        kvw_pool = g.enter_context(tc.tile_pool(name="kvw", bufs=1))
        a_pool = g.enter_context(tc.tile_pool(name="a_pool", bufs=1))
        small = g.enter_context(tc.tile_pool(name="small", bufs=8))

        cexp_t = singles.tile([P, S], _dt_bf16)
        sexp_t = singles.tile([P, S], _dt_bf16)
        perm_t = singles.tile([P, P], _dt_bf16)
        ident_t = singles.tile([P, P], _dt_bf16)
        maskT_t = singles.tile([P, P], _dt_f32)

        kt_t = kv_pool.tile([P, NKV_L, S], _dt_bf16)  # K^T, roped
        v_t = kv_pool.tile([P, 16, NKV_L, VW], _dt_bf16)  # V natural + ones col
        kvw_t = kvw_pool.tile([P, KO, 512], _dt_bf16)  # K cols 0:256, V 256:512
        a_t = a_pool.tile([P, 16, NH_L * DH], _dt_bf16)

        # ones column for PV row sums (col 128 of every (sb, g) stripe)
        nc.vector.memset(v_t[:, :, :, DH : DH + 1], 1.0)

        cp_flip = [0]

        def cp(out, in_):
            if cp_flip[0] % 2 == 0:
                nc.scalar.copy(out=out, in_=in_)
            else:
                nc.vector.tensor_copy(out=out, in_=in_)
            cp_flip[0] += 1

        with ExitStack() as c1:
            xt_pool = c1.enter_context(tc.tile_pool(name="xt", bufs=1))
            wq_pool = c1.enter_context(tc.tile_pool(name="wqp", bufs=1))
            qt_pool = c1.enter_context(tc.tile_pool(name="qtp", bufs=2))
            pt_pool = c1.enter_context(tc.tile_pool(name="ptp", bufs=2))
            rope_pool = c1.enter_context(tc.tile_pool(name="rope", bufs=2))
            ps_mm = c1.enter_context(tc.tile_pool(name="ps_mm", bufs=2, space="PSUM"))
            ps_sw = c1.enter_context(tc.tile_pool(name="ps_sw", bufs=1, space="PSUM"))
            # [128, 2, 512] f32 = 2 banks per buf; full-block scores pairs share
            # one tile so exp() runs once per 1024 columns.
            ps_sc = c1.enter_context(tc.tile_pool(name="ps_sc", bufs=2, space="PSUM"))
            ps_av = c1.enter_context(tc.tile_pool(name="ps_av", bufs=1, space="PSUM"))

            def rope_chunk(dst, psrc, s_off, w):
                """dst (sbuf bf16 [128, w]) <- rope(psrc (psum f32 [128, w])).
                gpsimd handles the cos-mul off the critical tail; the final
                add is on vector (fast) since consumers wait on dst.  All ops
                off scalar so exp never contends."""
                nc.vector.tensor_copy(out=dst, in_=psrc)
                sw = ps_sw.tile([P, 512], _dt_f32, tag="sw")
                nc.tensor.matmul(
                    sw[:, :w], lhsT=perm_t, rhs=dst, start=True, stop=True
                )
                # bufs=1: vector serializes t1 use in-order anyway; t2's WAR
                # only couples gpsimd behind the previous vector add (slack ok)
                t1 = rope_pool.tile([P, 512], _dt_bf16, tag="t1", bufs=1)
                t2 = rope_pool.tile([P, 512], _dt_bf16, tag="t2", bufs=1)
                nc.gpsimd.tensor_mul(t2[:, :w], dst, cexp_t[:, s_off : s_off + w])
                nc.vector.tensor_mul(t1[:, :w], sw[:, :w], sexp_t[:, s_off : s_off + w])
                nc.vector.tensor_add(dst, t2[:, :w], t1[:, :w])

            def dma_xt_granule(xt_half, hi, sblk, kq):
                # xt_half is the (tagged) [P, KO, 512] tile for sc = sblk // 2
                sb2 = sblk % 2
                nc.sync.dma_start(
                    out=xt_half[
                        :, kq * 8 : (kq + 1) * 8, sb2 * 256 : (sb2 + 1) * 256
                    ],
                    in_=xr[
                        :,
                        kq * 8 : (kq + 1) * 8,
                        hi * SH + sblk * 256 : hi * SH + (sblk + 1) * 256,
                    ],
                )

            def dma_wq(hp):
                # per-head tiles: the first filler chain only waits on 1.05MB
                tiles = []
                for h2 in range(2):
                    wq_t = wq_pool.tile(
                        [P, KO, 128], _dt_bf16, tag=f"wq{h2}", bufs=1
                    )
                    for i in range(2):
                        nc.sync.dma_start(
                            out=wq_t[:, i * 16 : (i + 1) * 16, :],
                            in_=wqr[
                                :,
                                i * 16 : (i + 1) * 16,
                                hp * 256 + h2 * 128 : hp * 256 + (h2 + 1) * 128,
                            ],
                        )
                    tiles.append(wq_t)
                return tiles

            def v_chain(xts, g, sbl, hi):
                # xts = (xtA, xtB); sbl 0..7 -> tile sbl//4, local col sbl%4
                # g is ignored: both kv heads projected in one 256-wide chain
                if g == 1:
                    return
                xth = xts[sbl // 4]
                sl = sbl % 4
                gv = ps_mm.tile([P, 512], _dt_f32, tag="mm")
                for ko in range(KO):
                    nc.tensor.matmul(
                        gv[:, :256],
                        lhsT=xth[:, ko, sl * P : (sl + 1) * P],
                        rhs=kvw_t[:, ko, 256:512],
                        start=(ko == 0),
                        stop=(ko == KO - 1),
                    )
                nc.vector.tensor_copy(
                    out=v_t[:, hi * 8 + sbl, :, 0:P], in_=gv[:, :256]
                )

            def q_chain(xts, wq_ts, qt, h2, sc, hi):
                gq = ps_mm.tile([P, 512], _dt_f32, tag="mm")
                for ko in range(KO):
                    nc.tensor.matmul(
                        gq,
                        lhsT=wq_ts[h2][:, ko, 0:P],
                        rhs=xts[sc][:, ko, :],
                        start=(ko == 0),
                        stop=(ko == KO - 1),
                    )
                rope_chunk(
                    qt[:, h2, sc * 512 : (sc + 1) * 512],
                    gq,
                    hi * SH + sc * 512,
                    512,
                )

            def k_chain(xts, g2, sc, hi):
                gk = ps_mm.tile([P, 512], _dt_f32, tag="mm")
                for ko in range(KO):
                    nc.tensor.matmul(
                        gk,
                        lhsT=kvw_t[:, ko, g2 * P : (g2 + 1) * P],
                        rhs=xts[sc][:, ko, :],
                        start=(ko == 0),
                        stop=(ko == KO - 1),
                    )
                rope_chunk(
                    kt_t[:, g2, hi * SH + sc * 512 : hi * SH + (sc + 1) * 512],
                    gk,
                    hi * SH + sc * 512,
                    512,
                )

            def scores_part(qt, h, h2, qcl, hi):
                """scT + exp for attn block (head h, q chunk qc=hi*2+qcl).
                Returns the P^T tile [128k, 16 kb slots, 512q]."""
                qc = hi * 2 + qcl
                g2 = h // 4
                nfull = 4 * qc
                # tag by qcl: same-tag blocks are 2 apart in the pipeline, so
                # bufs=1 per tag still double-buffers consecutive blocks.
                pt = pt_pool.tile(
                    [P, 12 + 4 * qcl, 512], _dt_bf16, tag=f"pt{qcl}", bufs=1
                )
                for kb2 in range(nfull // 2):
                    # paired full blocks: one exp over 1024 columns
                    ssc = ps_sc.tile([P, 2, 512], _dt_f32, tag="sc")
                    for u in range(2):
                        nc.tensor.matmul(
                            ssc[:, u, :],
                            lhsT=kt_t[
                                :, g2, (2 * kb2 + u) * P : (2 * kb2 + u + 1) * P
                            ],
                            rhs=qt[:, h2, qcl * 512 : (qcl + 1) * 512],
                            start=True,
                            stop=True,
                        )
                    nc.scalar.activation(
                        out=pt[:, 2 * kb2 : 2 * kb2 + 2, :],
                        in_=ssc,
                        func=mybir.ActivationFunctionType.Exp,
                        scale=SCALE,
                    )
                for jp in range(2):
                    ssc = ps_sc.tile([P, 2, 512], _dt_f32, tag="sc")
                    for u in range(2):
                        j = 2 * jp + u
                        kb = nfull + j
                        w = 512 - j * P
                        nc.tensor.matmul(
                            ssc[:, u, :w],
                            lhsT=kt_t[:, g2, kb * P : (kb + 1) * P],
                            rhs=qt[:, h2, qcl * 512 + j * P : (qcl + 1) * 512],
                            start=True,
                            stop=True,
                        )
                        nc.vector.tensor_add(
                            ssc[:, u, 0:P], ssc[:, u, 0:P], maskT_t
                        )
                        nc.scalar.activation(
                            out=pt[:, kb, j * P : 512],
                            in_=ssc[:, u, :w],
                            func=mybir.ActivationFunctionType.Exp,
                            scale=SCALE,
                        )
                return pt

            def pv_part(pt, h, qcl, hi):
                qc = hi * 2 + qcl
                g2 = h // 4
                for jq in range(4):
                    qi = qc * 4 + jq
                    nkb = 4 * qc + jq + 1
                    pa = ps_av.tile([P, 132], _dt_f32, tag="pa")
                    for kb in range(nkb):
                        nc.tensor.matmul(
                            pa[:, 0:129],
                            lhsT=pt[:, kb, jq * P : (jq + 1) * P],
                            rhs=v_t[:, kb, g2, 0:129],
                            start=(kb == 0),
                            stop=(kb == nkb - 1),
                        )
                    rinv = small.tile([P, 1], _dt_f32, tag="r")
                    nc.vector.reciprocal(rinv, pa[:, DH : DH + 1])
                    nc.vector.tensor_scalar_mul(
                        a_t[:, qi, h * P : (h + 1) * P], pa[:, 0:P], rinv
                    )

            # ---- startup DMAs (half 0), in consumption-arrival order ----
            xt0A = xt_pool.tile([P, KO, 512], _dt_bf16, tag="xtA", bufs=1)
            xt0B = xt_pool.tile([P, KO, 512], _dt_bf16, tag="xtB", bufs=1)
            for kq in range(4):
                # both V weight heads + x sblk0, ko-granule interleaved
                nc.sync.dma_start(
                    out=kvw_t[:, kq * 8 : (kq + 1) * 8, 256:512],
                    in_=wkvr[:, kq * 8 : (kq + 1) * 8, 256:512],
                )
                dma_xt_granule(xt0A, 0, 0, kq)
            for kq in range(4):
                dma_xt_granule(xt0A, 0, 1, kq)
            wq0 = dma_wq(0)
            nc.sync.dma_start(out=cexp_t, in_=aps["cexp"])
            nc.sync.dma_start(out=sexp_t, in_=aps["sexp"])
            nc.sync.dma_start(out=perm_t, in_=aps["perm"])
            for sblk in (2, 3):
                for kq in range(4):
                    dma_xt_granule(xt0B, 0, sblk, kq)
            for i in range(2):  # K weights
                nc.sync.dma_start(
                    out=kvw_t[:, i * 16 : (i + 1) * 16, 0:256],
                    in_=wkvr[:, i * 16 : (i + 1) * 16, 0:256],
                )
            nc.sync.dma_start(out=ident_t, in_=aps["ident"])
            nc.sync.dma_start(out=maskT_t, in_=aps["maskT"])

            state = {"xts": (xt0A, xt0B), "wq0": wq0}

            for hi in range(2):
                xts = state["xts"]
                wq_t = state["wq0"] if hi == 0 else state["wq0_h1"]
                qt = qt_pool.tile([P, 2, SH], _dt_bf16, tag="qt")
                # ---- V/Q/K projections in DMA-arrival order ----
                for sp in range(2):
                    for sbl in range(4):
                        if hi == 1 and sp == 0 and sbl < 2:
                            continue  # done as hp3-half0 fillers
                        for gv in range(NKV_L):
                            v_chain(xts, gv, sp * 4 + sbl, hi)
                    for h2 in range(2):
                        q_chain(xts, wq_t, qt, h2, sp, hi)
                for g2 in range(NKV_L):
                    for sc in range(2):
                        k_chain(xts, g2, sc, hi)

                def t_group_inplace(sb):
                    """Transpose a_t[:, sb] 128x128 blocks in place (A -> A^T
                    layout for wo).  PE filler for the last head-pair of half
                    1, where no Q-proj chains are available."""
                    for hb4 in (0, 4):
                        tp = ps_sw.tile([P, 512], _dt_bf16, tag="sw")
                        for j in range(4):
                            nc.tensor.transpose(
                                tp[:, j * P : (j + 1) * P],
                                a_t[:, sb, (hb4 + j) * P : (hb4 + j + 1) * P],
                                ident_t,
                            )
                        nc.vector.tensor_copy(
                            out=a_t[:, sb, hb4 * P : (hb4 + 4) * P], in_=tp
                        )

                # ---- attention, scores 2-ahead, next-hp Q proj interleaved ----
                for hp in range(4):
                    fillers = [None] * 4
                    if hp == 3 and hi == 0:
                        nxt = (state["xt1A"], None)
                        vb = [(0, 0), (1, 0), (0, 1), (1, 1)]
                        for i in range(4):
                            fillers[i] = (
                                lambda gv=vb[i][0], sbl=vb[i][1]:
                                v_chain(nxt, gv, sbl, 1)
                            )
                    if hp == 3 and hi == 1:
                        tb = [(0,), (1, 2), (3, 4), (5, 6, 7)]
                        for i in range(4):
                            fillers[i] = (
                                lambda sbs=tb[i]: [t_group_inplace(s) for s in sbs]
                            )
                    if hp < 3:
                        wq_n = dma_wq(hp + 1)
                        qt_n = qt_pool.tile([P, 2, SH], _dt_bf16, tag="qt")
                        # sc=0 chains first so xtB (and next half's xtA) free early
                        for i, (h2, sc) in enumerate(
                            [(0, 0), (1, 0), (0, 1), (1, 1)]
                        ):
                            fillers[i] = (
                                lambda h2=h2, sc=sc, wq_n=wq_n, qt_n=qt_n:
                                q_chain(xts, wq_n, qt_n, h2, sc, hi)
                            )

                    blocks = [(0, 0), (0, 1), (1, 0), (1, 1)]
                    pts = [None] * 4

                    def sc_i(i):
                        h2, qcl = blocks[i]
                        pts[i] = scores_part(qt, hp * 2 + h2, h2, qcl, hi)

                    def pv_i(i):
                        h2, qcl = blocks[i]
                        pv_part(pts[i], hp * 2 + h2, qcl, hi)

                    def f_i(i):
                        if fillers[i] is not None:
                            fillers[i]()

                    # scores run 2 ahead; fillers pad exp latency before PVs
                    sc_i(0); f_i(0); sc_i(1); pv_i(0); sc_i(2); f_i(1)
                    if hp == 2 and hi == 0:
                        # seam: next half's xtA + hp0 weights (WAR clears
                        # once this half's sc=0 readers retire)
                        xt1A = xt_pool.tile(
                            [P, KO, 512], _dt_bf16, tag="xtA", bufs=1
                        )
                        for sblk in (0, 1):
                            for kq in range(4):
                                dma_xt_granule(xt1A, 1, sblk, kq)
                        wq0_h1 = dma_wq(0)
                        state["wq0_h1"] = wq0_h1
                        state["xt1A"] = xt1A
                    pv_i(1); sc_i(3); f_i(2); pv_i(2); f_i(3); pv_i(3)
                    if hp == 3 and hi == 0:
                        xt1B = xt_pool.tile(
                            [P, KO, 512], _dt_bf16, tag="xtB", bufs=1
                        )
                        for sblk in (2, 3):
                            for kq in range(4):
                                dma_xt_granule(xt1B, 1, sblk, kq)
                        state["xts"] = (state["xt1A"], xt1B)
                    if hp < 3:
                        qt = qt_n

        # ---- wo projection: out[s, e] = sum_hd A^T[hd, sb].T @ woT[hd, e] ----
        with ExitStack() as c2:
            wo_pool = c2.enter_context(tc.tile_pool(name="wop", bufs=1))
            at_pool = c2.enter_context(tc.tile_pool(name="atp", bufs=2))
            ostage = c2.enter_context(tc.tile_pool(name="ostage", bufs=2))
            ps_at = c2.enter_context(tc.tile_pool(name="ps_at", bufs=2, space="PSUM"))
            ps_mo = c2.enter_context(tc.tile_pool(name="ps_mo", bufs=2, space="PSUM"))

            wo_t = wo_pool.tile([P, NH_L, D], _dt_bf16)
            for i in range(4):
                nc.sync.dma_start(
                    out=wo_t[:, i * 2 : (i + 1) * 2, :],
                    in_=wor[:, i * 2 : (i + 1) * 2, :],
                )

            def transpose_group(sb):
                at = at_pool.tile([P, NH_L, P], _dt_bf16, tag="at")
                for hb4 in range(0, NH_L, 4):
                    tp4 = ps_at.tile([P, 4, P], _dt_bf16, tag="at4")
                    for j in range(4):
                        nc.tensor.transpose(
                            tp4[:, j, :],
                            a_t[:, sb, (hb4 + j) * P : (hb4 + j + 1) * P],
                            ident_t,
                        )
                    cp(at[:, hb4 : hb4 + 4, :], tp4)
                return at

            at_cur = None
            for sb in range(16):
                ost = ostage.tile([P, D], _dt_bf16, tag="ost")
                for ec in range(8):
                    go = ps_mo.tile([P, 512], _dt_f32, tag="mo")
                    for hb in range(NH_L):
                        # sb < 8: a_t was transposed in place during half 1
                        lhs = (
                            a_t[:, sb, hb * P : (hb + 1) * P]
                            if sb < 8
                            else at_cur[:, hb, :]
                        )
                        nc.tensor.matmul(
                            go,
                            lhsT=lhs,
                            rhs=wo_t[:, hb, ec * 512 : (ec + 1) * 512],
                            start=(hb == 0),
                            stop=(hb == NH_L - 1),
                        )
                    cp(ost[:, ec * 512 : (ec + 1) * 512], go)
                    if sb == 15:
                        # chunked DMA on the last row so the kernel tail is
                        # one copy + one small DMA
                        nc.sync.dma_start(
                            out=out_ap[
                                sb * P : (sb + 1) * P, ec * 512 : (ec + 1) * 512
                            ],
                            in_=ost[:, ec * 512 : (ec + 1) * 512],
                        )
                    if ec == 0 and 7 <= sb < 15:
                        at_nxt = transpose_group(sb + 1)
                if sb < 15:
                    nc.sync.dma_start(
                        out=out_ap[sb * P : (sb + 1) * P, :], in_=ost
                    )
                    if 7 <= sb < 15:
                        at_cur = at_nxt


def _split_excess_waits(nc, limit=1):
    """Hoist excess semaphore waits onto same-engine no-ops inserted
    immediately before the offending instruction."""
    ctr = [0]
    for bb in nc.main_func.blocks:
        insts = list(bb.instructions)
        out = []
        changed = False
        for ins in insts:
            si = ins.sync_info
            waits = list(si.on_wait) if si and si.on_wait else []
            if len(waits) > limit:
                keep = waits[:limit]
                rest = waits[limit:]
                for i in range(0, len(rest), limit):
                    nop = mybir.InstNoOp(name=f"I-waitsplit-{ctr[0]}", ins=[], outs=[])
                    ctr[0] += 1
                    nop.engine = ins.engine
                    nop.sync_info = mybir.SyncInfo(
                        on_wait=rest[i : i + limit], on_update=[]
                    )
                    nc.register_instruction(nop)
                    out.append(nop)
                si.on_wait = keep
                changed = True
            out.append(ins)
        if changed:
            bb.instructions = out
    return ctr[0]


_PROGRAM_CACHE = {}


def build_program():
    if "nc" in _PROGRAM_CACHE:
        return _PROGRAM_CACHE["nc"]
    nc = bass.Bass("TRN2", target_bir_lowering=False, debug=False, num_devices=N_CORES)
    aps = {
        "xT": nc.dram_tensor("xT", [D, S], _dt_bf16, kind="ExternalInput").ap(),
        "wqT": nc.dram_tensor("wqT", [D, NH_L * DH], _dt_bf16, kind="ExternalInput").ap(),
        "wkvT": nc.dram_tensor("wkvT", [D, 512], _dt_bf16, kind="ExternalInput").ap(),
        "woT": nc.dram_tensor("woT", [NH_L * DH, D], _dt_bf16, kind="ExternalInput").ap(),
        "cexp": nc.dram_tensor("cexp", [P, S], _dt_bf16, kind="ExternalInput").ap(),
        "sexp": nc.dram_tensor("sexp", [P, S], _dt_bf16, kind="ExternalInput").ap(),
        "perm": nc.dram_tensor("perm", [P, P], _dt_bf16, kind="ExternalInput").ap(),
        "ident": nc.dram_tensor("ident", [P, P], _dt_bf16, kind="ExternalInput").ap(),
        "maskT": nc.dram_tensor("maskT", [P, P], _dt_f32, kind="ExternalInput").ap(),
        "out": nc.dram_tensor("out", [S, D], _dt_bf16, kind="ExternalOutput").ap(),
    }
    with tile.TileContext(nc) as tc:
        _emit(tc, aps)
    _split_excess_waits(nc, limit=1)
    _PROGRAM_CACHE["nc"] = nc
    return nc


def make_in_maps(x, freqs_cos, freqs_sin, mask, wq, wk, wv, wo):
    x = np.asarray(x, np.float32)
    freqs_cos = np.asarray(freqs_cos, np.float32)
    freqs_sin = np.asarray(freqs_sin, np.float32)
    mask = np.asarray(mask, np.float32)
    wq = np.asarray(wq, np.float32)
    wk = np.asarray(wk, np.float32)
    wv = np.asarray(wv, np.float32)
    wo = np.asarray(wo, np.float32)

    xb = [x[b].T.astype(BF16) for b in range(2)]  # (4096, 2048)
    cexp = np.repeat(freqs_cos.T, 2, axis=0).astype(BF16)  # (128, 2048)
    sx = np.repeat(freqs_sin.T, 2, axis=0).astype(np.float32)
    sx[0::2] *= -1.0
    sexp = sx.astype(BF16)
    perm = np.zeros((P, P), np.float32)
    idx = np.arange(P)
    perm[idx, idx ^ 1] = 1.0
    perm = perm.astype(BF16)
    ident = np.eye(P, dtype=np.float32).astype(BF16)
    maskT = np.ascontiguousarray(mask[:P, :P].T, dtype=np.float32)

    in_maps = []
    for core in range(N_CORES):
        b, tp = core // 4, core % 4
        wqT = wq[tp * 1024 : (tp + 1) * 1024].T.astype(BF16)  # (4096, 1024)
        wkT = wk[tp * 256 : (tp + 1) * 256].T.astype(BF16)  # (4096, 256)
        wvT = wv[tp * 256 : (tp + 1) * 256].T.astype(BF16)
        wkvT = np.ascontiguousarray(np.concatenate([wkT, wvT], axis=1))
        woT = wo[:, tp * 1024 : (tp + 1) * 1024].T.astype(BF16)  # (1024, 4096)
        in_maps.append(
            {
                "xT": xb[b],
                "wqT": wqT,
                "wkvT": wkvT,
                "woT": woT,
                "cexp": cexp,
                "sexp": sexp,
                "perm": perm,
                "ident": ident,
                "maskT": maskT,
            }
        )
    return in_maps


def run(inputs, trace=False):
    nc = build_program()
    in_maps = make_in_maps(
        inputs["x"],
        inputs["freqs_cos"],
        inputs["freqs_sin"],
        inputs["mask"],
        inputs["wq"],
        inputs["wk"],
        inputs["wv"],
        inputs["wo"],
    )
    res = run_bass_kernel_spmd(nc, in_maps, list(range(N_CORES)), trace=trace)
    out = np.zeros((2, S, D), np.float32)
    for core in range(N_CORES):
        out[core // 4] += np.asarray(res.results[core]["out"]).astype(np.float32)
    return out, res


def kernel(x, freqs_cos, freqs_sin, positions, mask, wq, wk, wv, wo):
    out, _ = run(
        {
            "x": x,
            "freqs_cos": freqs_cos,
            "freqs_sin": freqs_sin,
            "mask": mask,
            "wq": wq,
            "wk": wk,
            "wv": wv,
            "wo": wo,
        }
    )
    return out
